# revision 31
# baseline (speedup 1.0000x reference)
# Trainium2 Bass kernel for nn_AttentionLayer (BiDAF-style attention).
#
# Math (T=16384, J=1024, D2=512):
#   w1,w2,w3 = Ws blocks;  S[t,j] = H@w1 + U@w2 + (H*w3)@U.T
#   A  = softmax_j(S) @ U                      (C2Q)
#   b  = softmax_t(max_j S);  h~ = b @ H       (Q2C, global over T)
#   G  = [H | A | H*A | H*h~]                  (T, 2048)
#
# Sharding: T rows split across 8 cores (2048 rows each). U/Ws replicated.
# Per core everything is local except (hnum = sum_t exp(m_t) H_t, ssum =
# sum_t exp(m_t)), exchanged via a single AllGather (one ring pass, ~7
# hops) + local sum -- measurably faster than AllReduce (two ring passes)
# since the tiny 2KB payload makes the collective pure hop latency.  A
# dummy PAIRWISE AllGather ([[0,1],[2,3],...]) on a NEFF-const input
# fires at kernel start: it absorbs the global first-collective barrier
# (which fires on the FIRST cc op regardless of replica group) but rings
# in ~4us instead of ~9, freeing the serial CC stream earlier for the
# real AllGather.  Chain observed: barrier ends ~57-68us, then a fixed
# ~11us CC-stream gap, dummy ~4us, ~2us gap, real ring 15-31us.
# NOTE the collective output must be shaped [1, 8, 520] (not [8, 520]):
# the gather-landing DMA into a single SBUF partition otherwise clobbers
# neighboring persistent tiles on partitions 1-7 on real hardware (the
# simulator normalizes the access pattern and hides it).
# Fleet-state caveat: the first-collective barrier (13-128us) and the
# real ring time (6-40us) are strongly correlated run-to-run -- a
# throttled peer core slows both; the fleet also runs with a chronic
# ~60% util throttle (throttle_active covers most of the kernel), so
# engine rates are well below nominal and run-to-run variance is large.
#
# Layout trick: compute S^T tiles [j_part, t_free] so the C2Q attend matmul
# (A = P @ U) can use E=exp(S^T) slices directly as the stationary operand.
# exp bias handles the s2[j] term (per-partition); the s1[t] term cancels in
# softmax_j and is reapplied only to the Q2C row maxima.
#
# Perf structure (PE is the bottleneck: ~2x65536 rows for the two big
# GEMMs; bf16 streams 1 column/cycle and FWL hides most LDWEIGHTS for
# full 128-col stationaries; fp8 DoubleRow measured-infeasible on
# ACCURACY: numpy sim gives rel 3.4e-2 vs the 2e-2 gate):
#  - everything downstream of the exps is bf16: G is a bf16 DRAM tensor
#    (host upconverts to f32), hn/bnum/a_sb/ha/hts/hh all bf16 -> halves
#    G-write DMA bytes (8.4MB vs 16.8MB/core) and doubles DVE rate.
#    Numpy sim for the all-bf16 pipeline: rel ~7e-3 (gate 2e-2).
#  - LOOKAHEAD EMISSION: phase2a_head(c) [pmax, s1, trc, emax, bnum]
#    lags the S matmuls by ONE chunk and phase2a_tail(c) [hnum MMs] by
#    TWO, so the in-order PE queue (only LDW pull-ahead) never
#    head-of-line blocks on cross-engine reduction round-trips (es1
#    sits behind the next chunk's exps on the strict-FIFO Scalar
#    queue, hence the extra hnum lag).  S went ~515 -> ~237ns/MM.
#  - s1[t] = H@w1 on DVE (hn*w1b mult + row-reduce) directly in
#    t-partition layout: no PE matmul, no transpose (w1 broadcast to
#    [128, D2] host-side, like w2).
#  - softmax denominator: bf16 tree adds on DVE PRE-trigger (chunks
#    0-2; the S-window DVE has slack; tree depth 3 keeps the rounding
#    ~0.6%), then post-trigger per-t-tile partition sums via 4 matmuls
#    into ONE [128,4,2] psum tile + one strided DVE copy out.  The "ta"
#    ring needs bufs=4 (tree results live until their dps consume them).
#  - S runs as two interleaved PSUM chains (same-bank accumulation
#    serializes the PE, measured 464 -> 300ns per LDW+MM); the attends
#    run FOUR chains by borrowing the idle S-phase "sps" banks (same
#    tag/size -> same ring slots).
#  - Q2C tail strictly AFTER the last attends: the AllGather result
#    [8,520] lands via the idle GpSimd queue (ANY earlier position on a
#    busy queue blocks that strict-FIFO queue on the AG; a Scalar-queue
#    slot before the a_sb acts measured a 9us PE stall, and PE gather
#    MMs before phase3(3) measured attends blocked until AG+14us on a
#    dying fleet).  Summed by ONE PE matmul (ones8 stationary) instead
#    of 7 serial DVE adds; 1/ssum folds into a DVE tensor_scalar_mul
#    (f32r matmul operands MUST be DVE-written: the BIR verifier
#    rejects Scalar-activation outputs as "not rounded to FP32r").
#    H*h~ splits GpSimd (4 tiles, ~1.3us each, idle engine) / DVE (12).
#  - PE warmup: 12 identb transposes during the load phase start the
#    p-state clock ramp (~3us of sustained work -> full clock).
#  - G block 0 (= H) is written back during the load phase: spreads the
#    G writes and thins DMA traffic under the collective ring.
#  - PSUM budget is exactly 8 banks: sps x3, aps x2, tr x1 (trc + the
#    post-trigger ssum gather, reusing the tag to avoid a 9th bank and a
#    dps-ring deadlock), row x1 (warmup + hnps + hgps), dps x1.
# Tile readiness is tracked PER-TILE: un split into two single-writer
# tiles (una/unb) so the first exps aren't gated on the second half's
# DMA.  Splitting ht/hn the same way measured WORSE (157 vs 138us) --
# split multi-writer tiles only when a reader provably stalls on an
# unrelated writer DMA.
# Known-bad variants (measured): fp8 S/attend matmuls (numpy sim rel
# 3.4e-2 vs the 2e-2 gate -- softmax weight noise doesn't average out
# enough); vector.tensor_tensor_reduce hard-crashes the exec unit;
# inlining attends before the trigger exposes the collective latency;
# DMA f32->f32r is a rejected "cast" (declare f32, .bitcast at use).

import numpy as np

T, J, D2 = 16384, 1024, 512
NCORES = 8
TC = T // NCORES            # 2048 context rows per core
NCHUNK = 4                  # t-chunks per core
CHUNK = TC // NCHUNK        # 512
NTT = TC // 128             # 16 t-tiles per core
NJT = J // 128              # 8 j-tiles
NKT = D2 // 128             # 4 d-tiles

_CACHE = {}
LAST = {}


def _build_nc():
    import concourse.bacc as bacc
    import concourse.mybir as mybir
    import concourse.tile as tile

    f32 = mybir.dt.float32
    f32r = mybir.dt.float32r
    bf16 = mybir.dt.bfloat16
    X = mybir.AxisListType.X
    MAX = mybir.AluOpType.max
    ADD = mybir.AluOpType.add
    MULT = mybir.AluOpType.mult
    BYP = mybir.AluOpType.bypass
    EXP = mybir.ActivationFunctionType.Exp
    CPY = mybir.ActivationFunctionType.Copy

    nc = bacc.Bacc("TRN2", target_bir_lowering=False, debug=False,
                   num_devices=NCORES)

    HT = nc.dram_tensor("HT", [D2, TC], bf16, kind="ExternalInput")
    Hn = nc.dram_tensor("Hn", [TC, D2], bf16, kind="ExternalInput")
    Un = nc.dram_tensor("Un", [J, D2], bf16, kind="ExternalInput")
    UW = nc.dram_tensor("UW", [D2, J], bf16, kind="ExternalInput")
    W1b = nc.dram_tensor("W1b", [128, D2], bf16, kind="ExternalInput")
    W2b = nc.dram_tensor("W2b", [128, D2], bf16, kind="ExternalInput")
    Ib = nc.dram_tensor("Ib", [128, 128], bf16, kind="ExternalInput")
    On = nc.dram_tensor("On", [1, 128], f32r, kind="ExternalInput")
    Oc = nc.dram_tensor("Oc", [128, 2], bf16, kind="ExternalInput")
    Or = nc.dram_tensor("Or", [128, 1], f32r, kind="ExternalInput")
    G = nc.dram_tensor("G", [TC, 4 * D2], bf16, kind="ExternalOutput")

    with tile.TileContext(nc) as tc:
        with (
            tc.tile_pool(name="persist", bufs=1) as pp,
            tc.tile_pool(name="stream", bufs=2) as sp,
            tc.tile_pool(name="stage", bufs=4) as gp,
            tc.tile_pool(name="hhpool", bufs=3) as hp,
            tc.tile_pool(name="epool", bufs=4) as ep,
            tc.tile_pool(name="spsum", bufs=3, space="PSUM") as spsum,
            tc.tile_pool(name="apsum", bufs=2, space="PSUM") as apsum,
            tc.tile_pool(name="trpsum", bufs=1, space="PSUM") as trpsum,
            tc.tile_pool(name="rowpsum", bufs=1, space="PSUM") as rowpsum,
            tc.tile_pool(name="dpsum", bufs=1, space="PSUM") as dpsum,
            tc.tile_pool(name="dram", bufs=1, space="DRAM") as dram,
        ):
            # ---- dummy collective first: pays the first-collective
            # warmup/barrier while the engines do real work.  Its input is a
            # NEFF-const DRAM tensor (loaded at model-load time), so the
            # trigger has zero kernel-time dependencies.
            # Pairwise groups: the dummy only needs to absorb the global
            # first-collective barrier; a 1-hop exchange frees the CC stream
            # ~6us earlier than a full 7-hop ring for the real AllGather.
            dummy_in = nc.inline_tensor(np.zeros((1, 520), np.float32),
                                        name="dummy_in")
            dummy_out = dram.tile([1, 2, 520], f32, tag="dummy_out")
            nc.gpsimd.collective_compute(
                "AllGather", BYP,
                replica_groups=[[2 * i, 2 * i + 1] for i in range(NCORES // 2)],
                ins=[dummy_in.ap()], outs=[dummy_out.opt()],
            )

            # ---- loads, in the order the pipeline consumes them:
            # S matmuls need uw3 + ht[chunk0]; the first exps need s2col,
            # which needs un + w2b.
            uw3 = pp.tile([128, NKT, J], bf16, tag="uw3")
            ht = pp.tile([128, NKT, TC], bf16, tag="ht")
            for kt in range(NKT):
                nc.sync.dma_start(
                    uw3[:, kt, :], UW.ap()[kt * 128:(kt + 1) * 128, :])
                nc.sync.dma_start(
                    ht[:, kt, 0:CHUNK],
                    HT.ap()[kt * 128:(kt + 1) * 128, 0:CHUNK])
            # un split into two single-writer tiles: a slice reader is
            # otherwise gated on BOTH half-loads, stalling the first exps
            # (s2 bias) ~3us behind the second un DMA
            # small tiles load via the SCALAR queue: the Sync queue's serial
            # issue of ~15 loads otherwise delays ht chunk 1 past S(1)'s
            # start and the first MM to ~6-8us.
            una = pp.tile([128, 4, D2], bf16, tag="una")
            unb = pp.tile([128, 4, D2], bf16, tag="unb")

            def unt(jt):
                return una[:, jt, :] if jt < 4 else unb[:, jt - 4, :]

            # identb first: it feeds the PE warmup transposes that start the
            # tensor engine's p-state ramp during the load phase (the first
            # ~3us of S matmuls otherwise run at the cold clock).
            identb = pp.tile([128, 128], bf16, tag="identb")
            nc.scalar.dma_start(identb[:], Ib.ap()[:])
            warm = rowpsum.tile([128, 128], bf16, tag="row", name="warm")
            for _ in range(12):
                nc.tensor.transpose(warm[:], identb[:], identb[:])
            w2b = pp.tile([128, D2], bf16, tag="w2b")
            nc.scalar.dma_start(
                una[:],
                Un.ap()[0:512, :].rearrange("(jt p) d -> p jt d", p=128))
            nc.scalar.dma_start(w2b[:], W2b.ap()[:])
            # NOTE: loading chunk-1 ht ahead of un[4:8] measured WORSE both
            # times it was tried (153-155 vs 145us): the delayed second un
            # half pushes the s2 biases for j-tiles 4-7, stalling the exps
            # of every chunk.  Keep un complete before any later ht chunk.
            nc.scalar.dma_start(
                unb[:],
                Un.ap()[512:1024, :].rearrange("(jt p) d -> p jt d", p=128))
            w1b = pp.tile([128, D2], bf16, tag="w1b")
            nc.scalar.dma_start(w1b[:], W1b.ap()[:])
            for c in range(1, NCHUNK):
                cs, ce = c * CHUNK, (c + 1) * CHUNK
                nc.sync.dma_start(
                    ht[:, :, cs:ce],
                    HT.ap()[:, cs:ce].rearrange("(kt p) t -> p kt t", p=128))
            hn = pp.tile([128, NTT, D2], bf16, tag="hn")
            for c in range(NCHUNK):
                cs, ce = c * CHUNK, (c + 1) * CHUNK
                nc.sync.dma_start(
                    hn[:, 4 * c:4 * (c + 1), :],
                    Hn.ap()[cs:ce, :].rearrange("(tt p) d -> p tt d", p=128))
                # G block 0 (= H) written back immediately: spreads the G
                # writes into the load/S phase and thins out the DMA fabric
                # during the AllGather ring + deferred attend phase.
                nc.sync.dma_start(
                    G.ap()[cs:ce, 0:D2].rearrange("(q p) d -> p q d", p=128),
                    hn[:, 4 * c:4 * (c + 1), :])
            onesrow = pp.tile([1, 128], f32r, tag="onesrow")
            nc.scalar.dma_start(onesrow[:], On.ap()[:])
            onescol = pp.tile([128, 2], bf16, tag="onescol")
            nc.scalar.dma_start(onescol[:], Oc.ap()[:])
            orcol = pp.tile([128, 1], f32r, tag="orcol")
            nc.scalar.dma_start(orcol[:], Or.ap()[:])

            # ---- s2[j] = U @ w2 on DVE: per-(j)-partition columns directly
            # all-bf16 multiply (DVE 2x mode), f32 accumulation in the reduce
            s2col = pp.tile([128, NJT], f32, tag="s2col")
            for jt in range(NJT):
                scr = gp.tile([128, D2], bf16, tag="ttscr")
                nc.vector.tensor_tensor(scr[:], unt(jt), w2b[:], MULT)
                nc.vector.tensor_reduce(s2col[:, jt:jt + 1], scr[:], X, ADD)

            # ---- persistent accumulators
            emax = pp.tile([128, NTT], f32, tag="emax")    # max_j E'' per t
            dcol = pp.tile([128, NTT, 1], f32, tag="dcol")  # sum_j E'' per t
            s1col = pp.tile([128, NTT], f32, tag="s1col")  # s1[t]
            es1 = pp.tile([128, NTT], f32, tag="es1")      # exp(s1[t])
            bnum = pp.tile([128, NTT], bf16, tag="bnum")   # exp(m[t])
            hnum_sb = pp.tile([1, D2], f32, tag="hnum_sb")  # sum_t bnum*H

            # collective payload row, zero-padded up front (off trigger path)
            arow = pp.tile([1, 520], f32, tag="arow")
            nc.vector.memset(arow[:], 0.0)

            def phase1(c):
                # S^T tiles -> E'' = exp(S^T + s2[j]), two interleaved chains
                cs, ce = c * CHUNK, (c + 1) * CHUNK
                e = ep.tile([128, NJT, CHUNK], bf16, tag="e", name=f"e_{c}")
                for jq in range(0, NJT, 2):
                    spss = [spsum.tile([128, CHUNK], f32, tag="sps",
                                       name=f"sps_{c}_{jq}_{q}")
                            for q in range(2)]
                    for kt in range(NKT):
                        for q in range(2):
                            nc.tensor.matmul(
                                spss[q][:],
                                uw3[:, kt, (jq + q) * 128:(jq + q + 1) * 128],
                                ht[:, kt, cs:ce],
                                start=(kt == 0), stop=(kt == NKT - 1))
                    for q in range(2):
                        nc.scalar.activation(e[:, jq + q, :], spss[q][:], EXP,
                                             bias=s2col[:, jq + q:jq + q + 1])
                return e

            def phase2a_head(c, e):
                # Q2C-critical reductions ONLY (pmax -> emax -> bnum feed the
                # collective trigger).  Emitted one chunk BEHIND the S
                # matmuls so the PE never waits on the DVE/Scalar hops; the
                # hnum matmuls lag one MORE chunk (phase2a_tail) because
                # bnum's es1 sits behind the next chunk's exps on the Scalar
                # queue.
                pmax = sp.tile([128, CHUNK], bf16, tag="pmax",
                               name=f"pmax_{c}")
                nc.vector.tensor_tensor(pmax[:], e[:, 0, :], e[:, 1, :], MAX)
                for jt in range(2, NJT):
                    nc.vector.tensor_tensor(pmax[:], pmax[:], e[:, jt, :],
                                            MAX)

                # s1[t] on DVE directly in t-partition layout (hn * w1b, row
                # sum): no PE matmul, no transpose, frees ~1.2us PE per chunk
                for i in range(4):
                    tt = 4 * c + i
                    scr2 = gp.tile([128, D2], bf16, tag="s1scr")
                    nc.vector.tensor_tensor(scr2[:], hn[:, tt, :], w1b[:],
                                            MULT)
                    nc.vector.tensor_reduce(s1col[:, tt:tt + 1], scr2[:], X,
                                            ADD)

                # pmax transposes batched on PE, then the reduce/exp/mult
                trc = trpsum.tile([128, 4, 128], bf16, tag="tr",
                                  name=f"trc_{c}")
                for i in range(4):
                    nc.tensor.transpose(trc[:, i, :],
                                        pmax[:, i * 128:(i + 1) * 128],
                                        identb[:])
                for i in range(4):
                    tt = 4 * c + i
                    nc.vector.tensor_reduce(emax[:, tt:tt + 1],
                                            trc[:, i, :], X, MAX)
                    nc.scalar.activation(es1[:, tt:tt + 1],
                                         s1col[:, tt:tt + 1], EXP)
                for i in range(4):
                    tt = 4 * c + i
                    # bnum = exp(m[t]) = emax * exp(s1)
                    nc.vector.tensor_tensor(bnum[:, tt:tt + 1],
                                            emax[:, tt:tt + 1],
                                            es1[:, tt:tt + 1], MULT)

            def phase2a_tail(c):
                # Q2C numerator: hnps += bnum_tile.T @ H_tile
                hnps = rowpsum.tile([1, D2], f32, tag="row", name=f"hnps_{c}")
                for i in range(4):
                    tt = 4 * c + i
                    nc.tensor.matmul(hnps[:], bnum[:, tt:tt + 1],
                                     hn[:, tt, :],
                                     start=(i == 0), stop=(i == 3))
                if c == 0:
                    nc.vector.tensor_copy(hnum_sb[:], hnps[:])
                else:
                    nc.vector.tensor_tensor(hnum_sb[:], hnum_sb[:], hnps[:],
                                            ADD)

            def q2c_trigger():
                # ssum = sum_t bnum[t];  exchange [hnum | ssum]
                ssps = rowpsum.tile([1, NTT], f32, tag="row", name="ssps")
                nc.tensor.matmul(ssps[:], onescol[:, 0:1], bnum[:],
                                 start=True, stop=True)
                nc.vector.tensor_copy(arow[0:1, 0:D2], hnum_sb[:])
                nc.vector.tensor_reduce(arow[0:1, D2:D2 + 1], ssps[:], X, ADD)
                # AllGather + local sum instead of AllReduce: one ring pass
                # (7 hops) instead of reduce-scatter + all-gather (14), and
                # the hops are what's slow under concurrent G-write DMA.
                ar_in = dram.tile([1, 520], f32, tag="ar_in")
                ar_out = dram.tile([1, NCORES, 520], f32, tag="ar_out")
                nc.sync.dma_start(ar_in[:], arow[:])
                nc.gpsimd.collective_compute(
                    "AllGather", BYP, replica_groups=[list(range(NCORES))],
                    ins=[ar_in.opt()], outs=[ar_out.opt()],
                )
                return ar_out

            psms = {}

            def phase2d_tree(c, e):
                # softmax denominator part 1: bf16 tree adds on DVE.  Runs
                # PRE-trigger for chunks 0-2 (the S-phase DVE has ~30us of
                # slack) so the attend-window DVE only carries ha/hh.  The
                # "ta" ring must hold all pending results until their
                # post-trigger dps matmuls consume them (bufs=4).
                t0_ = tp.tile([128, CHUNK], bf16, tag="ta", bufs=4,
                              name=f"ta_{c}")
                t1_ = tp.tile([128, CHUNK], bf16, tag="tb", name=f"tb_{c}")
                t2_ = tp.tile([128, CHUNK], bf16, tag="tc", name=f"tc_{c}")
                t3_ = tp.tile([128, CHUNK], bf16, tag="td", name=f"td_{c}")
                nc.vector.tensor_tensor(t0_[:], e[:, 0, :], e[:, 1, :], ADD)
                nc.vector.tensor_tensor(t1_[:], e[:, 2, :], e[:, 3, :], ADD)
                nc.vector.tensor_tensor(t2_[:], e[:, 4, :], e[:, 5, :], ADD)
                nc.vector.tensor_tensor(t3_[:], e[:, 6, :], e[:, 7, :], ADD)
                nc.vector.tensor_tensor(t0_[:], t0_[:], t1_[:], ADD)
                nc.vector.tensor_tensor(t2_[:], t2_[:], t3_[:], ADD)
                nc.vector.tensor_tensor(t0_[:], t0_[:], t2_[:], ADD)
                psms[c] = t0_

            def phase2d_dps(c):
                # part 2 (post-trigger): partition sums via 4 matmuls into
                # one psum tile, one strided DVE copy out.
                t0_ = psms[c]
                dpsc = dpsum.tile([128, 4, 2], f32, tag="dps",
                                  name=f"dps_{c}")
                for i in range(4):
                    nc.tensor.matmul(dpsc[:, i, :],
                                     t0_[:, i * 128:(i + 1) * 128],
                                     onescol[:], start=True, stop=True)
                nc.vector.tensor_copy(dcol[:, 4 * c:4 * (c + 1), :],
                                      dpsc[:, :, 0:1])

            def phase3(c, e):
                # C2Q attend + G blocks 1..2, FOUR interleaved PSUM chains:
                # the S-phase "sps" banks are idle post-trigger, so chains
                # 2-3 borrow them (same tag, same size -> same ring slots).
                apss = [apsum.tile([128, D2], f32, tag="aps",
                                   name=f"aps_{c}_{q}") for q in range(2)]
                apss += [spsum.tile([128, D2], f32, tag="sps",
                                    name=f"apss_{c}_{q}") for q in range(2)]
                for jt in range(NJT):
                    for q in range(4):
                        nc.tensor.matmul(
                            apss[q][:],
                            e[:, jt, q * 128:(q + 1) * 128],
                            unt(jt),
                            start=(jt == 0), stop=(jt == NJT - 1))
                if True:
                    for q in range(4):
                        tt = 4 * c + q
                        dinv = gp.tile([128, 1], f32, tag="dinv")
                        nc.vector.reciprocal(dinv[:], dcol[:, tt, :])
                        # A = psum * (1/d) on the Scalar engine (frees DVE).
                        # NOTE: staging [A | H*A] in one tile with a single
                        # combined DMA measured WORSE (154 vs 145us at equal
                        # fleet state): it delays the A bytes behind the H*A
                        # compute, and Sync issue cost was not the gate.
                        a_sb = gp.tile([128, D2], bf16, tag="a_sb")
                        nc.scalar.activation(a_sb[:], apss[q][:], CPY,
                                             scale=dinv[:])
                        ha_sb = gp.tile([128, D2], bf16, tag="ha_sb")
                        nc.vector.tensor_tensor(ha_sb[:], hn[:, tt, :],
                                                a_sb[:], MULT)
                        ts_, te_ = tt * 128, (tt + 1) * 128
                        nc.sync.dma_start(G.ap()[ts_:te_, D2:2 * D2],
                                          a_sb[:])
                        nc.sync.dma_start(G.ap()[ts_:te_, 2 * D2:3 * D2],
                                          ha_sb[:])

            def q2c_hts(hg8b):
                # local sum of the 8 gathered partial rows via ONE PE matmul
                # (ones8 stationary) -- replaces 7 serial DVE adds.  ssum
                # lands in the trpsum bank (free post-trigger; a dpsum tag
                # would deadlock against the dps ring).
                hgps = rowpsum.tile([1, D2], f32, tag="row", name="hgps")
                nc.tensor.matmul(hgps[:], orcol[0:8, :],
                                 hg8b[:, 0:D2].bitcast(f32r),
                                 start=True, stop=True)
                ssg = trpsum.tile([1, 8], f32, tag="tr", name="ssg")
                nc.tensor.matmul(ssg[:], orcol[0:8, :],
                                 hg8b[:, D2:D2 + 8].bitcast(f32r),
                                 start=True, stop=True)
                zinv = pp.tile([1, 1], f32, tag="zinv")
                nc.vector.reciprocal(zinv[:], ssg[0:1, 0:1])
                # 1/ssum folded into the psum->sbuf copy of the gathered row,
                # so hg IS h~ and no per-partition zcol broadcast is needed.
                # (Must be a DVE write: the compiler rejects Scalar-activation
                # output consumed as f32r by a matmul -- not f32r-rounded.)
                hg = pp.tile([1, D2], f32r, tag="hg")
                nc.vector.tensor_scalar_mul(hg[:], hgps[:], zinv[0:1, 0:1])
                htps = apsum.tile([128, D2], f32, tag="aps", name="htps")
                nc.tensor.matmul(htps[:], onesrow[:], hg[0:1, 0:D2],
                                 start=True, stop=True)
                hts = pp.tile([128, D2], bf16, tag="hts")
                nc.vector.tensor_copy(hts[:], htps[:])
                return hts

            def g3_write(rs, ntile, tile_):
                # G3 writes from the Scalar queue: bypasses the Sync queue's
                # backlog of attend writes.
                nc.scalar.dma_start(
                    G.ap()[rs:rs + ntile * 128, 3 * D2:4 * D2]
                    .rearrange("(q p) d -> p q d", p=128),
                    tile_[:])

            with tc.tile_pool(name="treepool", bufs=2) as tp:
                # ---- pre-trigger: S matmuls stream with the Q2C reductions
                # lagging one chunk behind and the hnum matmuls two behind
                # (lookahead emission keeps the in-order PE queue stall-free).
                es = []
                for c in range(NCHUNK):
                    es.append(phase1(c))
                    if c >= 1:
                        phase2a_head(c - 1, es[c - 1])
                        phase2d_tree(c - 1, es[c - 1])
                    if c >= 2:
                        phase2a_tail(c - 2)
                phase2a_head(NCHUNK - 1, es[NCHUNK - 1])
                phase2a_tail(NCHUNK - 2)
                phase2a_tail(NCHUNK - 1)
                ar_out = q2c_trigger()

                # ---- post-trigger: denominator transposes one chunk ahead
                # of the attends (the tree sums already ran pre-trigger).
                phase2d_tree(3, es[3])
                phase2d_dps(0)
                phase2d_dps(1)
                phase3(0, es[0])
                phase2d_dps(2)
                phase3(1, es[1])
                phase2d_dps(3)
                phase3(2, es[2])
                phase3(3, es[3])
                # land the AllGather payload via the idle GpSimd queue,
                # emitted only now: ANY earlier engine-queue position blocks
                # that strict-FIFO queue on the AG completion (a Scalar-queue
                # slot before the a_sb acts measured a 9us PE stall: the
                # chunk-3 attends' psum recycle waits on those acts).
                hg8b = pp.tile([8, 520], f32, tag="hg8b")
                nc.gpsimd.dma_start(
                    hg8b[:], ar_out[:].rearrange("o p d -> (o p) d"))
                # Q2C tail strictly AFTER the last attends: sandwiching the
                # gather matmuls before phase3(3) head-of-line blocked the
                # chunk-3 attends behind the AllGather on a slow fleet
                # (measured: attends(3) ran 172-186us after a 107us barrier).
                hts = q2c_hts(hg8b)
                # G block 3: H * h~.  GpSimd (idle, ~1.7us/tile) takes the
                # chunk-3 tiles in parallel with the DVE's 12 (bf16 2x,
                # ~0.42us/tile); writes issue per group as they finish.
                hhg = hp.tile([128, 4, D2], bf16, tag="hhg", name="hhg")
                for k in range(4):
                    nc.gpsimd.tensor_tensor(hhg[:, k, :], hn[:, 12 + k, :],
                                            hts[:], MULT)
                for g in range(6):
                    hh = hp.tile([128, 2, D2], bf16, tag="hh2",
                                 name=f"hh_{g}")
                    for k in range(2):
                        tt = 2 * g + k
                        nc.vector.tensor_tensor(hh[:, k, :], hn[:, tt, :],
                                                hts[:], MULT)
                    g3_write(g * 256, 2, hh)
                g3_write(12 * 128, 4, hhg)

    nc.compile()
    return nc


def kernel(H, U, Ws):
    import concourse.mybir as mybir
    from concourse import bass_utils

    H = np.ascontiguousarray(np.asarray(H, dtype=np.float32))
    U = np.ascontiguousarray(np.asarray(U, dtype=np.float32))
    Ws = np.asarray(Ws, dtype=np.float32)

    if "nc" not in _CACHE:
        _CACHE["nc"] = _build_nc()
    nc = _CACHE["nc"]

    bfnp = mybir.dt.np(mybir.dt.bfloat16)

    w1 = Ws[0:D2, 0]
    w2 = Ws[D2:2 * D2, 0]
    w3 = Ws[2 * D2:3 * D2, 0]
    UW = np.ascontiguousarray(U.T * w3[:, None]).astype(bfnp)
    Unc = U.astype(bfnp)
    W1b = np.ascontiguousarray(np.broadcast_to(w1, (128, D2))).astype(bfnp)
    W2b = np.ascontiguousarray(np.broadcast_to(w2, (128, D2))).astype(bfnp)
    ident = np.eye(128, dtype=np.float32)

    in_maps = []
    for c in range(NCORES):
        Hc = H[c * TC:(c + 1) * TC]
        in_maps.append({
            "HT": np.ascontiguousarray(Hc.T).astype(bfnp),
            "Hn": Hc.astype(bfnp),
            "Un": Unc,
            "UW": UW,
            "W1b": W1b,
            "W2b": W2b,
            "Ib": ident.astype(bfnp),
            "On": np.ones((1, 128), dtype=np.float32),
            "Oc": np.ones((128, 2), dtype=bfnp),
            "Or": np.ones((128, 1), dtype=np.float32),
        })

    res = bass_utils.run_bass_kernel_spmd(
        nc, in_maps, core_ids=list(range(NCORES)))
    LAST["exec_time_ns"] = res.exec_time_ns
    G_full = np.concatenate([res.results[c]["G"] for c in range(NCORES)],
                            axis=0)
    return G_full.astype(np.float32)
